# revision 3
# baseline (speedup 1.0000x reference)
"""Optimized Trainium2 Bass kernel for nn_DFTParallelRecon.

Polar-grid DFT CT reconstruction: sgm [2,1,512,512] -> rec [2,1,512,512].

Decomposition (same geometry as baseline): 16 work units = 2 BC x 4 bands x
2 frames, two classes (A: bands {0,3}, B: bands {1,2}) of 8 units each.
One merged SPMD program runs both classes with interleaved phases on 8 cores.

vs baseline:
  - bf16 data through the sampling stage (V, compact, outbox, inbox, fk):
    halves every gpsimd scatter and index table.
  - factored bilinear weights: V = (Pa[s] + wx*Da[s]) + wy*(E0[s] + wx*E1[s])
    with dense difference arrays Da/E0/E1 - only wx/wy tables needed.
  - C-major compact layout: sc2 reads each compact element ~once; outbox
    slots are global per (w, C) (no piece splitting).
  - sc3 is a single whole-row scatter per (class, C) frame (bf16 makes the
    destination fit the 2046-element gpsimd cap) - no range inflation.
  - f32r stage-1 matmuls, bf16 stage-3 matmuls.
"""
import sys
sys.path.insert(0, '/opt/trn_rl_repo')
import numpy as np
import ml_dtypes

BF16 = ml_dtypes.bfloat16

FM = 2048; V = 512; FN = 2048; M = 512; N_DET = 512; A_DET = 1.0; PIX = 0.5
TWO_PI = 2.0 * np.pi
C0 = 512; NB = 1024; BAND = 256
CHUNK = 128; NCH = 2
DST_CAP = 2046        # local_scatter num_elems cap (< 2048, even)
NP_V = 4              # V/weight pieces per class
ABLATE = set()        # dev-only: {'scatter','vops','tabdma'}


# ---------------- geometry ----------------

def polar_pix():
    """All nonzero pixels with reference-exact f32 bilinear data."""
    v = ((np.arange(FM, dtype=np.float32) - np.float32((FM - 1) / 2))
         * np.float32(1.0 / (FM * PIX))).astype(np.float32)
    ky, kx = np.meshgrid(v, v, indexing='xy')
    mk = np.abs(kx + 1j * ky).astype(np.float32)
    th = np.arctan2(ky, kx).astype(np.float32)
    theta = ((th + np.float32(TWO_PI)) / np.float32(TWO_PI) * np.float32(V)).astype(np.float32)
    k = (mk * np.float32(FN * A_DET) + np.float32((FN - 1) / 2)).astype(np.float32)
    theta_norm = ((theta - np.float32(V)) / np.float32(V)).astype(np.float32)
    k_norm = ((k - np.float32((FN - 1) / 2)) / np.float32(FN // 2)).astype(np.float32)
    ix = ((k_norm + np.float32(1.0)) * np.float32(0.5) * np.float32(FN - 1)).astype(np.float32)
    iy = ((theta_norm + np.float32(1.0)) * np.float32(0.5) * np.float32(2 * V - 1)).astype(np.float32)
    x0 = np.floor(ix).astype(np.int64); y0 = np.floor(iy).astype(np.int64)
    wx1 = (ix - x0).astype(np.float32); wy1 = (iy - y0).astype(np.float32)
    I, J = np.meshgrid(np.arange(FM), np.arange(FM), indexing='ij')
    c = (FM - 1) / 2.0
    u = I - c; w = J - c
    inA = np.abs(u) >= np.abs(w)
    v0 = (x0 >= 0) & (x0 <= FN - 1)
    v1 = (x0 + 1 >= 0) & (x0 + 1 <= FN - 1)
    sel = v0 | v1
    ii, jj = np.nonzero(sel)
    # The factored weight form needs the x0 tap always in range; holds since
    # k >= 1023.5 here. The x0+1 tap going out of range is handled by the
    # zero-padded P column (exactly equivalent to the reference's masking).
    assert v0[ii, jj].all()
    P = {}
    P['i'] = ii; P['j'] = jj
    P['frame'] = (~inA[ii, jj]).astype(np.int8)
    P['Y'] = y0[ii, jj]; P['x0'] = x0[ii, jj]
    P['wx'] = wx1[ii, jj]; P['wy'] = wy1[ii, jj]
    return P


def core_pix(P, r, f, mirror):
    """Pixel arrays for core (band r, frame f): local coords.
    mirror=True relabels fi -> BAND-1-fi, fj -> NB-1-fj so that the mirror
    band's table ranges align with its partner's; G tables absorb the flip."""
    frow = np.where(P['frame'] == 0, P['i'], P['j'])
    fcol = np.where(P['frame'] == 0, P['j'], P['i'])
    sel = (P['frame'] == f) & (frow >= C0 + r * BAND) & (frow < C0 + (r + 1) * BAND)
    d = {}
    for k in ('Y', 'x0', 'wx', 'wy'):
        d[k] = P[k][sel]
    d['fi'] = (frow[sel] - C0 - r * BAND).astype(np.int64)
    d['fj'] = (fcol[sel] - C0).astype(np.int64)
    d['mirror'] = mirror
    if mirror:
        d['fi'] = (BAND - 1) - d['fi']
        d['fj'] = (NB - 1) - d['fj']
    Ys = np.unique(d['Y'])
    assert len(Ys) <= NCH * CHUNK, f"{len(Ys)} wedges"
    ypos = {y: i for i, y in enumerate(Ys)}
    g = np.array([ypos[y] for y in d['Y']], np.int64)
    d['ch'] = g // CHUNK
    d['part'] = g % CHUNK
    d['Ylist'] = np.concatenate([Ys, np.full(NCH * CHUNK - len(Ys), Ys[-1])]).reshape(NCH, CHUNK)
    d['n'] = len(d['Y'])
    d['q'] = d['fi'] % CHUNK
    d['C'] = d['fi'] // CHUNK
    return d


def runlen(key_sorted):
    """Rank within runs of equal keys (keys must be grouped)."""
    n = len(key_sorted)
    if n == 0:
        return np.zeros(0, np.int64)
    same = np.concatenate([[False], key_sorted[1:] == key_sorted[:-1]])
    idx = np.arange(n)
    prev = np.maximum.accumulate(np.where(~same, idx, 0))
    return idx - prev


def rank_by(order_keys, cell_key):
    """Rank of each element within its cell; elements ordered by order_keys.
    order_keys: tuple for np.lexsort (last key most significant); cell_key
    must be a function of the most-significant keys so cells are grouped."""
    order = np.lexsort(order_keys)
    rk = np.zeros(len(cell_key), np.int64)
    rk[order] = runlen(cell_key[order])
    return rk


class ClassMeta:
    pass


def build_class(P, bands):
    cores = []
    for r in bands:
        for f in (0, 1):
            cores.append((r, f, core_pix(P, r, f, mirror=(r >= 2))))
    S = ClassMeta()
    S.bands = bands
    S.xlo = min(int(d['x0'].min()) for _, _, d in cores)
    xext = max(int(d['x0'].max()) for _, _, d in cores) - S.xlo + 1
    S.NP = NP_V
    S.pw = -(-xext // S.NP)
    S.xpad = S.pw * S.NP
    S.xw1 = S.xpad + 1          # P columns [xlo, xlo+xpad]

    for _, _, d in cores:
        d['s'] = d['x0'] - S.xlo
        d['pc'] = d['s'] // S.pw
        d['sl'] = d['s'] - d['pc'] * S.pw
        # V slot k within (ch, part, pc, sl)
        cell = ((d['ch'] * CHUNK + d['part']) * S.NP + d['pc']) * S.pw + d['sl']
        d['k'] = rank_by((d['fj'], d['sl'], d['part'], d['ch'], d['pc']), cell)

    # K per (w, pc) maxed over cores of the class
    S.K = np.ones((NCH, S.NP), np.int64)
    for _, _, d in cores:
        for w in range(NCH):
            for pc in range(S.NP):
                m = (d['ch'] == w) & (d['pc'] == pc)
                if m.any():
                    S.K[w, pc] = max(S.K[w, pc], int(d['k'][m].max()) + 1)
    # single-weight-table units: K*pw elements per piece (no ri doubling)
    S.vw_off = np.zeros((NCH, S.NP), np.int64)
    S.vw_w = 0
    for w in range(NCH):
        cur = 0
        for pc in range(S.NP):
            S.vw_off[w, pc] = cur
            cur += S.K[w, pc] * S.pw
        S.vw_w = max(S.vw_w, cur)

    # compact: piece-major, [pc: C0-sub | C1-sub] per w. One sc1 scatter
    # covers a whole piece (dst = the piece segment, both C sub-blocks).
    for _, _, d in cores:
        cell = (((d['ch'] * CHUNK + d['part']) * S.NP + d['pc']) * 2 + d['C'])
        d['crk'] = rank_by((d['k'], d['sl'], d['C'], d['pc'], d['part'], d['ch']), cell)
    S.CPC = np.zeros((NCH, 2, S.NP), np.int64)
    for _, _, d in cores:
        for w in range(NCH):
            for C in (0, 1):
                for pc in range(S.NP):
                    m = (d['ch'] == w) & (d['C'] == C) & (d['pc'] == pc)
                    if m.any():
                        S.CPC[w, C, pc] = max(S.CPC[w, C, pc], int(d['crk'][m].max()) + 1)
    # piece segment start + C-sub offsets; C-order within a piece chosen per
    # w to minimize the total sc2 span (first C in the piece = S.cord[w][0])
    S.pstart = np.zeros((NCH, S.NP), np.int64)     # start of piece segment
    S.cpx = 0
    for w in range(NCH):
        cur = 0
        for pc in range(S.NP):
            S.pstart[w, pc] = cur
            cur += S.CPC[w, 0, pc] + S.CPC[w, 1, pc]
            assert (S.CPC[w, 0, pc] + S.CPC[w, 1, pc]) * 2 <= DST_CAP
        S.cpx = max(S.cpx, cur)
    S.cord = []
    S.sc2span = np.zeros((NCH, 2, 2), np.int64)    # (w, C) -> (lo, hi)
    for w in range(NCH):
        best = None
        for order in ((0, 1), (1, 0)):
            spans = {}
            tot = 0
            for C in (0, 1):
                lo, hi = None, None
                for pc in range(S.NP):
                    if S.CPC[w, C, pc] > 0:
                        st = S.pstart[w, pc] + (
                            S.CPC[w, order[0], pc] if C == order[1] else 0)
                        if lo is None:
                            lo = st
                        hi = st + S.CPC[w, C, pc]
                if lo is None:
                    lo, hi = 0, 1
                spans[C] = (lo, hi)
                tot += (hi - lo) * int(S.NG[w, C]) if hasattr(S, 'NG') else (hi - lo)
            if best is None or tot < best[0]:
                best = (tot, order, spans)
        _, order, spans = best
        S.cord.append(order)
        for C in (0, 1):
            S.sc2span[w, C] = spans[C]

    # sc1 sections: one per (w, pc)
    S.sc1_off = {}
    cur = 0
    for w in range(NCH):
        for pc in range(S.NP):
            S.sc1_off[(w, pc)] = cur
            cur += S.pw * S.K[w, pc] * 2
    S.sc1_w = cur

    # outbox slots: global per (w, C); rank within (w, part, C, q) by (s, k)
    for _, _, d in cores:
        cell = (((d['ch'] * CHUNK + d['part']) * 2 + d['C']) * CHUNK + d['q'])
        d['m'] = rank_by((d['k'], d['s'], d['q'], d['C'], d['part'], d['ch']), cell)
    S.Tn = np.ones((NCH, 2), np.int64)
    for _, _, d in cores:
        for w in range(NCH):
            for C in (0, 1):
                m = (d['ch'] == w) & (d['C'] == C)
                if m.any():
                    S.Tn[w, C] = max(S.Tn[w, C], int(d['m'][m].max()) + 1)
    # q segments per (w, C) so dst fits cap
    S.NG = np.ones((NCH, 2), np.int64)
    for w in range(NCH):
        for C in (0, 1):
            qn = CHUNK
            while qn * S.Tn[w, C] * 2 > DST_CAP:
                qn //= 2
            S.NG[w, C] = CHUNK // qn
    # sc2 sections: per (w, C, g): scans the (w, C) span
    S.sc2_off = {}
    cur = 0
    for w in range(NCH):
        for C in (0, 1):
            lo, hi = S.sc2span[w, C]
            for g in range(S.NG[w, C]):
                S.sc2_off[(w, C, g)] = cur
                cur += int(hi - lo) * 2
    S.sc2_w = cur

    # inbox layout per C: sections per w of CHUNK*Tn*2
    S.woff = np.zeros((2, NCH), np.int64)
    S.inbox_w = np.zeros(2, np.int64)
    for C in (0, 1):
        cur = 0
        for w in range(NCH):
            S.woff[C, w] = cur
            cur += CHUNK * int(S.Tn[w, C]) * 2
        S.inbox_w[C] = cur
    S.sc3_off = [0, int(S.inbox_w[0])]
    S.sc3_w = int(S.inbox_w[0] + S.inbox_w[1])

    # stage-3 column trim
    qlo = 8; qhi = 0
    for _, _, d in cores:
        qlo = min(qlo, int(d['fj'].min()) // CHUNK)
        qhi = max(qhi, int(d['fj'].max()) // CHUNK + 1)
    S.qlo, S.qhi = qlo, qhi
    S.nq = qhi - qlo
    assert S.nq * CHUNK * 2 <= DST_CAP

    tabs = {}
    for r, f, d in cores:
        tabs[(r, f)] = emit_core(d, S, r)
    return S, tabs


# ---------------- stage-1/3 matrices ----------------

def stage1_C():
    dx = A_DET; dk = 1.0 / (FN * dx)
    x0 = -(N_DET - 1) / 2 * dx; k0 = -(FN - 1) / 2 * dk
    m32 = np.arange(N_DET, dtype=np.float32)
    n32 = np.arange(FN, dtype=np.float32)
    ph_pre = (np.float32(TWO_PI * (k0 * dx)) * m32).astype(np.float32)
    pre = np.exp(-1j * ph_pre.astype(np.float64))
    inner = (np.float32(dk) * n32 + np.float32(k0)).astype(np.float32)
    ph_post = (np.float32(TWO_PI * x0) * inner).astype(np.float32)
    post = dx * np.exp(-1j * ph_post.astype(np.float64))
    mm = np.arange(N_DET, dtype=np.float64)
    nn = np.arange(FN, dtype=np.float64)
    Wm = np.exp(-1j * TWO_PI * np.outer(mm, nn) / FN)
    return ((pre[:, None] * Wm) * post[None, :]).astype(np.complex64)


def stage3_G():
    dx = PIX; dk = 1.0 / (FM * dx)
    x0 = -(FM - 1) / 2 * dx; k0 = -(FM - 1) / 2 * dk
    ar32 = np.arange(FM, dtype=np.float32)
    ph_pre = (np.float32(TWO_PI * (x0 * dk)) * ar32).astype(np.float32)
    pre = np.exp(1j * ph_pre.astype(np.float64))
    inner = (np.float32(dx) * ar32 + np.float32(x0)).astype(np.float32)
    ph_post = (np.float32(TWO_PI * k0) * inner).astype(np.float32)
    post = np.exp(1j * ph_post.astype(np.float64))
    lo = (FM - M) // 2
    p = np.arange(lo, lo + M)
    mm = np.arange(C0, C0 + NB)
    G = (dk * post[p])[:, None] * np.exp(1j * TWO_PI * np.outer(p, mm) / FM) * pre[mm][None, :]
    return G.astype(np.complex64)


_CM = None; _G = None


def emit_core(d, S, r):
    global _CM, _G
    if _CM is None:
        _CM = stage1_C(); _G = stage3_G()
    mirror = d['mirror']
    t = {}
    t['viewA'] = (d['Ylist'] % V).astype(np.int32)
    t['viewB'] = ((d['Ylist'] + 1) % V).astype(np.int32)
    ch, part, pc, sl, kk = d['ch'], d['part'], d['pc'], d['sl'], d['k']
    crk, mm_, q, C_, fj = d['crk'], d['m'], d['q'], d['C'], d['fj']

    # weight table wxy: per (w, pc) section [wx: K*pw | wy: K*pw], K-major
    # (slot (sl, k) at position k*pw + sl). V layout is [p, K, 2(ri), pw].
    wxy = np.zeros((NCH, CHUNK, 2 * S.vw_w), BF16)
    for w in range(NCH):
        for p in range(S.NP):
            m = (ch == w) & (pc == p)
            if not m.any():
                continue
            off = 2 * S.vw_off[w, p]; K = S.K[w, p]
            base = kk[m] * S.pw + sl[m]
            wxy[w, part[m], off + base] = d['wx'][m].astype(BF16)
            wxy[w, part[m], off + K * S.pw + base] = d['wy'][m].astype(BF16)
    t['wxy'] = wxy

    # sc1 idx: per (w, pc): whole V piece scanned once (flat (k, ri, sl));
    # dst position = (C*CPC0 + crk)*2 + h within the piece segment
    sc1 = np.full((CHUNK, S.sc1_w), -1, np.int16)
    for w in range(NCH):
        for p in range(S.NP):
            off = S.sc1_off[(w, p)]; K = S.K[w, p]
            m = (ch == w) & (pc == p)
            if not m.any():
                continue
            first = S.cord[w][0]
            loc = np.where(C_[m] == first, 0, S.CPC[w, first, p]) + crk[m]
            for h in range(2):
                src = (kk[m] * 2 + h) * S.pw + sl[m]
                sc1[part[m], off + src] = (loc * 2 + h).astype(np.int16)
    t['sc1'] = sc1

    # sc2 idx: per (w, C, g): scans the (w, C) span of compact
    sc2 = np.full((CHUNK, S.sc2_w), -1, np.int16)
    first = np.array([S.cord[w][0] for w in range(NCH)])[ch]
    cp = S.pstart[ch, pc] + np.where(C_ == first, 0, S.CPC[ch, first, pc]) + crk
    for w in range(NCH):
        for C in (0, 1):
            ng = S.NG[w, C]; qn = CHUNK // ng
            Tn = S.Tn[w, C]
            lo, hi = S.sc2span[w, C]
            m = (ch == w) & (C_ == C)
            if not m.any():
                continue
            for g in range(ng):
                off = S.sc2_off[(w, C, g)]
                mg = m & (q // qn == g)
                if not mg.any():
                    continue
                src = (cp[mg] - lo) * 2
                dst = ((q[mg] - g * qn) * Tn + mm_[mg]) * 2
                for h in range(2):
                    sc2[part[mg], off + src + h] = (dst + h).astype(np.int16)
    t['sc2'] = sc2

    # sc3 idx per C: one scatter, src = whole inbox, dst = fk row (trimmed)
    sc3 = np.full((CHUNK, S.sc3_w), -1, np.int16)
    for C in (0, 1):
        off = S.sc3_off[C]
        m = C_ == C
        w_ = ch[m]; p_ = part[m]; q_ = q[m]; fj_ = fj[m]
        ipos = S.woff[C, w_] + (p_ * S.Tn[w_, C] + mm_[m]) * 2
        dst = (fj_ - S.qlo * CHUNK) * 2
        assert (fj_ // CHUNK >= S.qlo).all() and (fj_ // CHUNK < S.qhi).all()
        for h in range(2):
            sc3[q_, off + ipos + h] = (dst + h).astype(np.int16)
    t['sc3'] = sc3

    # stage-1 C window, ri-major: [512, 2, xw1]
    xwin = np.zeros((N_DET, S.xw1), np.complex64)
    hi = min(S.xlo + S.xw1, FN)
    xwin[:, :hi - S.xlo] = _CM[:, S.xlo:hi]
    t['cmat'] = np.ascontiguousarray(
        np.stack([xwin.real, xwin.imag], 1).astype(BF16))

    # stage-3 G tables (bf16; gr ri-major)
    fi_loc = np.arange(BAND)
    if mirror:
        fi_loc = (BAND - 1) - fi_loc
    rows = r * BAND + fi_loc
    Gr = _G[:, rows].T                       # [256 fi, 512 a]
    t['grA'] = np.ascontiguousarray(
        np.stack([Gr.real, Gr.imag], 1).astype(BF16))        # [256, 2, 512]
    t['grB'] = np.ascontiguousarray(
        np.stack([-Gr.imag, Gr.real], 1).astype(BF16))
    fj_loc = np.arange(S.qlo * CHUNK, S.qhi * CHUNK)
    if mirror:
        fj_loc = (NB - 1) - fj_loc
    Gq = _G.T[fj_loc]                        # [nq*128, 512]
    t['gcA'] = np.ascontiguousarray(Gq.real.astype(BF16))
    t['gcB'] = np.ascontiguousarray((-Gq.imag).astype(BF16))
    return t


def build_all():
    P = polar_pix()
    SA, tabsA = build_class(P, (0, 3))
    SB, tabsB = build_class(P, (1, 2))
    return (SA, tabsA), (SB, tabsB)


# ---------------- host mock (device semantics, bf16) ----------------

def bf(x):
    return np.asarray(x, BF16)


def local_scatter_np(dst16, data16, idx16):
    dst16[:] = 0
    for prt in range(dst16.shape[0]):
        ii = idx16[prt]
        msk = ii >= 0
        dst16[prt, ii[msk].astype(np.int64)] = data16[prt, np.nonzero(msk)[0]]


def mock_core(sgm_b, t, S):
    f32 = np.float32
    sgm_b = np.asarray(sgm_b, BF16).astype(f32)
    Cm = t['cmat'][:, 0].astype(f32) + 1j * t['cmat'][:, 1].astype(f32)
    compacts = []
    for w in range(NCH):
        Pa = sgm_b[t['viewA'][w]].astype(np.complex64) @ Cm   # [128, xw1]
        Pb = sgm_b[t['viewB'][w]].astype(np.complex64) @ Cm
        Pa16 = np.stack([bf(Pa.real), bf(Pa.imag)], 1)        # [128, 2, xw1]
        Pb16 = np.stack([bf(Pb.real), bf(Pb.imag)], 1)
        E0f = bf(Pb16.astype(f32) - Pa16.astype(f32))
        Da = bf(Pa16[:, :, 1:].astype(f32) - Pa16[:, :, :-1].astype(f32))
        E1 = bf(E0f[:, :, 1:].astype(f32) - E0f[:, :, :-1].astype(f32))
        compact = np.zeros((CHUNK, S.cpx * 2), np.int16)
        for p in range(S.NP):
            K = S.K[w, p]; off = 2 * S.vw_off[w, p]
            a = p * S.pw; b = a + S.pw
            wxp = t['wxy'][w][:, off:off + K * S.pw].reshape(CHUNK, K, 1, S.pw)
            wyp = t['wxy'][w][:, off + K * S.pw:off + 2 * K * S.pw].reshape(CHUNK, K, 1, S.pw)
            t1 = bf(wxp.astype(f32) * E1[:, None, :, a:b].astype(f32))
            t2 = bf(t1.astype(f32) + E0f[:, None, :, a:b].astype(f32))
            t3 = bf(wxp.astype(f32) * Da[:, None, :, a:b].astype(f32))
            t4 = bf(t3.astype(f32) + Pa16[:, None, :, a:b].astype(f32))
            t5 = bf(wyp.astype(f32) * t2.astype(f32))
            Vv = bf(t4.astype(f32) + t5.astype(f32))     # [128, K, 2, pw]
            V16 = np.ascontiguousarray(Vv).view(np.int16).reshape(CHUNK, -1)
            o1 = S.sc1_off[(w, p)]
            idx = t['sc1'][:, o1:o1 + S.pw * K * 2]
            cs = S.pstart[w, p]
            cc = S.CPC[w, 0, p] + S.CPC[w, 1, p]
            seg = compact[:, cs * 2:(cs + cc) * 2]
            local_scatter_np(seg, V16, idx)
        compacts.append(compact)
    # sc2 -> outbox [p, q, Tn, 2] per (w, C)
    obx = {}
    for w in range(NCH):
        for C in (0, 1):
            Tn = S.Tn[w, C]
            ob = np.zeros((CHUNK, CHUNK * Tn * 2), np.int16)
            ng = S.NG[w, C]; qn = CHUNK // ng
            lo, hi = S.sc2span[w, C]
            for g in range(ng):
                off = S.sc2_off[(w, C, g)]
                idx = t['sc2'][:, off:off + (hi - lo) * 2]
                seg = ob[:, g * qn * Tn * 2:(g + 1) * qn * Tn * 2]
                local_scatter_np(seg, compacts[w][:, lo * 2:(hi) * 2], idx)
            obx[(w, C)] = ob.reshape(CHUNK, CHUNK, Tn, 2)
    # transpose -> inbox, sc3 -> fk
    fkt = {}
    for C in (0, 1):
        inbox = np.zeros((CHUNK, int(S.inbox_w[C])), np.int16)
        for w in range(NCH):
            Tn = S.Tn[w, C]
            wof = S.woff[C, w]
            blk = obx[(w, C)]
            for m in range(Tn):
                for h in range(2):
                    inbox[:, wof + (np.arange(CHUNK) * Tn + m) * 2 + h] = \
                        blk[:, :, m, h].T
        off = S.sc3_off[C]
        idx = t['sc3'][:, off:off + int(S.inbox_w[C])]
        fk = np.zeros((CHUNK, S.nq * CHUNK * 2), np.int16)
        local_scatter_np(fk, inbox, idx)
        fkt[C] = fk.view(BF16).reshape(CHUNK, S.nq * CHUNK, 2)
    # stage 3 (f32 psum accumulate of bf16 products)
    Tq = np.zeros((S.nq, CHUNK, 2, 512), BF16)
    for qc in range(S.nq):
        for ri in range(2):
            acc = np.zeros((CHUNK, 512), f32)
            for C in (0, 1):
                gA = t['grA'][C * CHUNK:(C + 1) * CHUNK, ri].astype(f32)
                gB = t['grB'][C * CHUNK:(C + 1) * CHUNK, ri].astype(f32)
                fkb = fkt[C][:, qc * CHUNK:(qc + 1) * CHUNK]
                acc += fkb[..., 0].astype(f32).T @ gA
                acc += fkb[..., 1].astype(f32).T @ gB
            Tq[qc, :, ri] = bf(acc)
    U = np.zeros((512, 512), f32)
    for bc in range(4):
        acc = np.zeros((CHUNK, 512), f32)
        for qc in range(S.nq):
            for comp, gname in ((0, 'gcA'), (1, 'gcB')):
                g = t[gname][qc * CHUNK:(qc + 1) * CHUNK,
                             bc * CHUNK:(bc + 1) * CHUNK].astype(f32)
                acc += g.T @ Tq[qc, :, comp].astype(f32)
        U[bc * CHUNK:(bc + 1) * CHUNK] = acc
    return U


def full_mock(sgm, classes):
    rec = np.zeros((2, M, M), np.float32)
    for (S, tabs) in classes:
        for (r, f), t in tabs.items():
            for b in range(2):
                out = mock_core(sgm[b, 0], t, S)
                rec[b] += out.T if f == 0 else out
    return rec.reshape(2, 1, M, M)


# ---------------- device program ----------------

import concourse.bass as bass
import concourse.mybir as mybir
from concourse import bacc
from concourse.tile import TileContext
from concourse.masks import make_identity

F32 = mybir.dt.float32
F32R = mybir.dt.float32r
BF = mybir.dt.bfloat16
I16 = mybir.dt.int16
MUL = mybir.AluOpType.mult
ADD = mybir.AluOpType.add
SUB = mybir.AluOpType.subtract


def build_program(SA, SB, repeat=1):
    """One merged SPMD program: class A and class B bodies, phase-interleaved."""
    nc = bacc.Bacc("TRN2", target_bir_lowering=False)
    cls = []
    for suf, S in (('A', SA), ('B', SB)):
        io = {}
        io['S'] = S
        io['suf'] = suf
        io['sgmTa'] = nc.dram_tensor(f"sgmTa{suf}", [NCH, N_DET, CHUNK], BF, kind="ExternalInput")
        io['sgmTb'] = nc.dram_tensor(f"sgmTb{suf}", [NCH, N_DET, CHUNK], BF, kind="ExternalInput")
        io['cmat'] = nc.dram_tensor(f"cmat{suf}", [N_DET, 2, S.xw1], BF, kind="ExternalInput")
        io['wxy'] = nc.dram_tensor(f"wxy{suf}", [NCH, CHUNK, 2 * S.vw_w], BF, kind="ExternalInput")
        io['sc1'] = nc.dram_tensor(f"sc1{suf}", [CHUNK, S.sc1_w], I16, kind="ExternalInput")
        io['sc2'] = nc.dram_tensor(f"sc2{suf}", [CHUNK, S.sc2_w], I16, kind="ExternalInput")
        io['sc3'] = nc.dram_tensor(f"sc3{suf}", [CHUNK, S.sc3_w], I16, kind="ExternalInput")
        io['grA'] = nc.dram_tensor(f"grA{suf}", [2 * CHUNK, 2, 512], BF, kind="ExternalInput")
        io['grB'] = nc.dram_tensor(f"grB{suf}", [2 * CHUNK, 2, 512], BF, kind="ExternalInput")
        io['gcA'] = nc.dram_tensor(f"gcA{suf}", [S.nq * CHUNK, 512], BF, kind="ExternalInput")
        io['gcB'] = nc.dram_tensor(f"gcB{suf}", [S.nq * CHUNK, 512], BF, kind="ExternalInput")
        io['out'] = nc.dram_tensor(f"out{suf}", [512, 512], F32, kind="ExternalOutput")
        cls.append(io)

    with TileContext(nc) as tc:
      import contextlib
      loop_cm = tc.For_i(0, repeat) if repeat > 1 else contextlib.nullcontext()
      with loop_cm:
        stk = contextlib.ExitStack()
        with stk:
            pools = {}
            for nm, bufs, space in (
                    ("const", 1, "SBUF"), ("s1p", 1, "SBUF"), ("ppool", 2, "SBUF"),
                    ("cpool", 1, "SBUF"), ("vst", 2, "SBUF"), ("vwk", 2, "SBUF"),
                    ("ist", 2, "SBUF"), ("ist1", 1, "SBUF"), ("obp", 1, "SBUF"),
                    ("ibp", 1, "SBUF"), ("fkp", 1, "SBUF"), ("s3p", 1, "SBUF"),
                    ("s3st", 2, "SBUF"), ("ps1", 2, "PSUM"), ("psT", 2, "PSUM"),
                    ("ps3", 2, "PSUM")):
                kw = {"space": space} if space == "PSUM" else {}
                pools[nm] = stk.enter_context(tc.tile_pool(name=nm, bufs=bufs, **kw))
            constp = pools["const"]; s1p = pools["s1p"]; ppool = pools["ppool"]
            cpool = pools["cpool"]; vst = pools["vst"]; vwk = pools["vwk"]
            ist = pools["ist"]; ist1 = pools["ist1"]; obp = pools["obp"]
            ibp = pools["ibp"]; fkp = pools["fkp"]; s3p = pools["s3p"]
            s3st = pools["s3st"]; ps1 = pools["ps1"]; psT = pools["psT"]
            ps3 = pools["ps3"]
            ident = constp.tile([CHUNK, CHUNK], BF)
            make_identity(nc, ident[:])

            # ---- stage 1 + V + sc1, fused per (class, w) ----
            for io in cls:
                S = io['S']
                cmt = s1p.tile([CHUNK, 4, 2, S.xw1], BF, tag="cm")
                nc.sync.dma_start(
                    out=cmt[:],
                    in_=io['cmat'].rearrange("(kc p) r x -> p kc r x", p=CHUNK))
                io['compact'] = {}
                for w in range(NCH):
                    sg = {}
                    for nm, dram in (("a", io['sgmTa']), ("b", io['sgmTb'])):
                        sgt = s1p.tile([CHUNK, 4, CHUNK], BF, tag=f"sg{nm}")
                        nc.sync.dma_start(
                            out=sgt[:],
                            in_=dram[w].rearrange("(kc p) c -> p kc c", p=CHUNK))
                        sg[nm] = sgt
                    Pa = ppool.tile([CHUNK, 2, S.xw1], BF, tag="Pa")
                    E0 = ppool.tile([CHUNK, 2, S.xw1], BF, tag="E0")
                    Da = ppool.tile([CHUNK, 2, S.xpad], BF, tag="Da")
                    E1 = ppool.tile([CHUNK, 2, S.xpad], BF, tag="E1")
                    ncopy = 0
                    for nm in ("a", "b"):
                        for ri in range(2):
                            xs = 0
                            while xs < S.xw1:
                                pl = min(512, S.xw1 - xs)
                                ps = ps1.tile([CHUNK, 512], F32, tag="s1")
                                for kc in range(4):
                                    nc.tensor.matmul(
                                        ps[:, :pl],
                                        sg[nm][:, kc, :],
                                        cmt[:, kc, ri, xs:xs + pl],
                                        start=(kc == 0), stop=(kc == 3))
                                if nm == "a":
                                    if ncopy % 2 == 0:
                                        nc.scalar.copy(out=Pa[:, ri, xs:xs + pl], in_=ps[:, :pl])
                                    else:
                                        nc.vector.tensor_copy(Pa[:, ri, xs:xs + pl], ps[:, :pl])
                                else:
                                    nc.vector.tensor_tensor(
                                        out=E0[:, ri, xs:xs + pl],
                                        in0=ps[:, :pl], in1=Pa[:, ri, xs:xs + pl], op=SUB)
                                ncopy += 1
                                xs += pl
                    nc.vector.tensor_tensor(out=Da[:], in0=Pa[:, :, 1:], in1=Pa[:, :, :-1], op=SUB)
                    nc.vector.tensor_tensor(out=E1[:], in0=E0[:, :, 1:], in1=E0[:, :, :-1], op=SUB)
                    # V + sc1 for this (class, w); V layout [p, K, 2, pw]
                    cpt = cpool.tile([CHUNK, S.cpx * 2], BF, tag=f"cpt{w}{io['suf']}")
                    io['compact'][w] = cpt
                    for p in range(S.NP):
                        K = int(S.K[w, p]); off = int(S.vw_off[w, p])
                        a = p * S.pw; b = a + S.pw
                        shp = (CHUNK, K, 2, S.pw)
                        wt = vst.tile([CHUNK, 2, K, S.pw], BF, tag="wxy")
                        if 'tabdma' in ABLATE:
                            nc.sync.dma_start(
                                out=wt[:, :, :, :1],
                                in_=io['wxy'][w, :, 2 * off:2 * off + 2 * K])
                        else:
                            nc.sync.dma_start(
                                out=wt[:],
                                in_=io['wxy'][w, :, 2 * off:2 * off + 2 * K * S.pw])
                        wxt = wt[:, 0, :, None, :].to_broadcast(shp)
                        wyt = wt[:, 1, :, None, :].to_broadcast(shp)
                        Vt = vwk.tile([CHUNK, K, 2, S.pw], BF, tag="V")
                        t2 = vwk.tile([CHUNK, K, 2, S.pw], BF, tag="t2")
                        E1b = E1[:, None, :, a:b].to_broadcast(shp)
                        E0b = E0[:, None, :, a:b].to_broadcast(shp)
                        Dab = Da[:, None, :, a:b].to_broadcast(shp)
                        Pab = Pa[:, None, :, a:b].to_broadcast(shp)
                        if 'vops' in ABLATE:
                            nc.vector.tensor_tensor(out=Vt[:], in0=wxt, in1=Dab, op=MUL)
                        else:
                            nc.vector.tensor_tensor(out=t2[:], in0=wxt, in1=E1b, op=MUL)
                            nc.vector.tensor_tensor(out=t2[:], in0=t2[:], in1=E0b, op=ADD)
                            nc.vector.tensor_tensor(out=Vt[:], in0=wxt, in1=Dab, op=MUL)
                            nc.vector.tensor_tensor(out=Vt[:], in0=Vt[:], in1=Pab, op=ADD)
                            nc.vector.tensor_tensor(out=t2[:], in0=wyt, in1=t2[:], op=MUL)
                            nc.vector.tensor_tensor(out=Vt[:], in0=Vt[:], in1=t2[:], op=ADD)
                        vflat = Vt[:].rearrange("p k r s -> p (k r s)")
                        cs = int(S.pstart[w, p])
                        cc = int(S.CPC[w, 0, p] + S.CPC[w, 1, p])
                        o1 = S.sc1_off[(w, p)]
                        idxt = vst.tile([CHUNK, S.pw * K * 2], I16, tag="sc1i")
                        if 'tabdma' in ABLATE:
                            nc.sync.dma_start(out=idxt[:, :2], in_=io['sc1'][:, o1:o1 + 2])
                        else:
                            nc.sync.dma_start(out=idxt[:], in_=io['sc1'][:, o1:o1 + S.pw * K * 2])
                        if 'scatter2' in ABLATE:
                            nc.vector.memset(cpt[:, cs * 2:(cs + cc) * 2], 0.0)
                        else:
                            nidx = 2 if 'scatter' in ABLATE else S.pw * K * 2
                            nc.gpsimd.local_scatter(
                                cpt[:, cs * 2:(cs + cc) * 2], vflat, idxt[:, :nidx],
                                channels=CHUNK, num_elems=cc * 2,
                                num_idxs=nidx)

            # ---- sc2 -> outbox, transpose -> inbox, sc3 -> fk (per class) ----
            for io in cls:
                S = io['S']
                io['ob'] = {}
                for w in range(NCH):
                    for C in (0, 1):
                        Tn = int(S.Tn[w, C])
                        ng = int(S.NG[w, C]); qn = CHUNK // ng
                        lo, hi = int(S.sc2span[w, C, 0]), int(S.sc2span[w, C, 1])
                        cwd = hi - lo
                        ob = obp.tile([CHUNK, CHUNK, Tn, 2], BF, tag=f"ob{w}{C}{io['suf']}")
                        io['ob'][(w, C)] = ob
                        obf = ob[:].rearrange("p q t r -> p (q t r)")
                        for g in range(ng):
                            off = S.sc2_off[(w, C, g)]
                            idxt = ist.tile([CHUNK, cwd * 2], I16, tag="sc2i")
                            if 'tabdma' in ABLATE:
                                nc.sync.dma_start(out=idxt[:, :2], in_=io['sc2'][:, off:off + 2])
                            else:
                                nc.sync.dma_start(out=idxt[:], in_=io['sc2'][:, off:off + cwd * 2])
                            if 'scatter2' in ABLATE:
                                nc.vector.memset(obf[:, g * qn * Tn * 2:(g + 1) * qn * Tn * 2], 0.0)
                            else:
                                nidx = 2 if 'scatter' in ABLATE else cwd * 2
                                nc.gpsimd.local_scatter(
                                    obf[:, g * qn * Tn * 2:(g + 1) * qn * Tn * 2],
                                    io['compact'][w][:, lo * 2:hi * 2], idxt[:, :nidx],
                                    channels=CHUNK, num_elems=qn * Tn * 2,
                                    num_idxs=nidx)

                # transpose -> inbox, sc3 -> fk (same class)
                S = io['S']
                io['fk'] = {}
                for C in (0, 1):
                    ib = ibp.tile([CHUNK, int(S.inbox_w[C])], BF, tag=f"ib{C}{io['suf']}")
                    ncp = 0
                    for w in range(NCH):
                        Tn = int(S.Tn[w, C])
                        wof = int(S.woff[C, w])
                        ob = io['ob'][(w, C)]
                        pairs = [(m, ri) for m in range(Tn) for ri in range(2)]
                        gi = 0
                        while gi < len(pairs):
                            grp = pairs[gi:gi + 4]
                            ps = psT.tile([CHUNK, 512], BF, tag="tp")
                            for j, (m, ri) in enumerate(grp):
                                nc.tensor.transpose(
                                    ps[:, j * CHUNK:(j + 1) * CHUNK],
                                    ob[:, :, m, ri], ident[:])
                            m0, r0 = grp[0]
                            j0 = m0 * 2 + r0
                            wview = ib[:, wof:wof + CHUNK * Tn * 2].rearrange(
                                "q (p t) -> q p t", t=Tn * 2)
                            dst_ap = wview[:, :, j0:j0 + len(grp)].rearrange(
                                "q p j -> q j p")
                            src_ap = ps[:, :len(grp) * CHUNK].rearrange(
                                "q (j p) -> q j p", j=len(grp))
                            nc.scalar.copy(out=dst_ap, in_=src_ap)
                            ncp += 1
                            gi += 4
                    fkt = fkp.tile([CHUNK, S.nq * CHUNK, 2], BF, tag=f"fk{C}{io['suf']}")
                    io['fk'][C] = fkt
                    off = S.sc3_off[C]
                    idxt = ist1.tile([CHUNK, int(S.inbox_w[C])], I16, tag="sc3i")
                    if 'tabdma' in ABLATE:
                        nc.sync.dma_start(out=idxt[:, :2], in_=io['sc3'][:, off:off + 2])
                    else:
                        nc.sync.dma_start(out=idxt[:], in_=io['sc3'][:, off:off + int(S.inbox_w[C])])
                    if 'scatter2' in ABLATE:
                        nc.vector.memset(fkt[:], 0.0)
                    else:
                        nidx = 2 if 'scatter' in ABLATE else int(S.inbox_w[C])
                        nc.gpsimd.local_scatter(
                            fkt[:].rearrange("p a r -> p (a r)"), ib[:], idxt[:, :nidx],
                            channels=CHUNK, num_elems=S.nq * CHUNK * 2,
                            num_idxs=nidx)

            # ---- stage 3 (both classes) ----
            for io in cls:
                S = io['S']
                io['grt'] = {}
                for nm in ('grA', 'grB'):
                    for C in (0, 1):
                        gt = s3p.tile([CHUNK, 2, 512], BF, tag=f"gr{nm}{C}")
                        nc.sync.dma_start(out=gt[:], in_=io[nm][C * CHUNK:(C + 1) * CHUNK])
                        io['grt'][(nm, C)] = gt
                Tt = []
                for qc in range(S.nq):
                    Tq = s3p.tile([CHUNK, 2, 512], BF, tag=f"T{qc}")
                    Tt.append(Tq)
                    for ri in range(2):
                        ps = ps3.tile([CHUNK, 512], F32, tag="s3t")
                        k = 0
                        for C in (0, 1):
                            for comp, gnm in ((0, "grA"), (1, "grB")):
                                nc.tensor.matmul(
                                    ps[:],
                                    io['fk'][C][:, qc * CHUNK:(qc + 1) * CHUNK, comp],
                                    io['grt'][(gnm, C)][:, ri, :],
                                    start=(k == 0), stop=(k == 3))
                                k += 1
                        nc.scalar.copy(out=Tq[:, ri, :], in_=ps[:])
                gcts = {}
                for qc in range(S.nq):
                    for comp, dram in ((0, io['gcA']), (1, io['gcB'])):
                        gct = s3p.tile([CHUNK, 512], BF, tag=f"gc{qc}{comp}")
                        nc.sync.dma_start(out=gct[:], in_=dram[qc * CHUNK:(qc + 1) * CHUNK])
                        gcts[(qc, comp)] = gct
                for bc in range(4):
                    ps = ps3.tile([CHUNK, 512], F32, tag="s3o")
                    k = 0
                    for qc in range(S.nq):
                        for comp in (0, 1):
                            nc.tensor.matmul(
                                ps[:], gcts[(qc, comp)][:, bc * CHUNK:(bc + 1) * CHUNK],
                                Tt[qc][:, comp, :],
                                start=(k == 0), stop=(k == 2 * S.nq - 1))
                            k += 1
                    ot = s3st.tile([CHUNK, 512], F32, tag="ot")
                    nc.any.tensor_copy(ot[:], ps[:])
                    nc.sync.dma_start(out=io['out'][bc * CHUNK:(bc + 1) * CHUNK], in_=ot[:])
    nc.compile()
    return nc


def core_inputs(classes, sgm):
    """in_maps for the 8 cores. Core order: (b, r, f) within each class."""
    ins = [dict() for _ in range(8)]
    for (S, tabs), suf in zip(classes, ('A', 'B')):
        i = 0
        for b in range(2):
            for r in S.bands:
                for f in (0, 1):
                    t = tabs[(r, f)]
                    im = ins[i]; i += 1
                    im[f'sgmTa{suf}'] = np.ascontiguousarray(
                        sgm[b, 0][t['viewA']].transpose(0, 2, 1).astype(BF16))
                    im[f'sgmTb{suf}'] = np.ascontiguousarray(
                        sgm[b, 0][t['viewB']].transpose(0, 2, 1).astype(BF16))
                    for nm in ('cmat', 'sc1', 'sc2', 'sc3', 'wxy',
                               'grA', 'grB', 'gcA', 'gcB'):
                        im[f'{nm}{suf}'] = t[nm]
    return ins


def combine_outputs(classes, results):
    rec = np.zeros((2, 512, 512), np.float32)
    for (S, tabs), suf in zip(classes, ('A', 'B')):
        i = 0
        for b in range(2):
            for r in S.bands:
                for f in (0, 1):
                    o = results[i][f'out{suf}']; i += 1
                    rec[b] += o.T if f == 0 else o
    return rec.reshape(2, 1, 512, 512)


_CACHE = {}


def _get_setup():
    if 'setup' not in _CACHE:
        classes = build_all()
        nc = build_program(classes[0][0], classes[1][0])
        _CACHE['setup'] = (classes, nc)
    return _CACHE['setup']


def kernel(sgm):
    from concourse.bass_utils import run_bass_kernel_spmd
    sgm = np.asarray(sgm, dtype=np.float32)
    assert sgm.shape == (2, 1, 512, 512)
    classes, nc = _get_setup()
    ins = core_inputs(classes, sgm)
    res = run_bass_kernel_spmd(nc, ins, core_ids=list(range(8)))
    return combine_outputs(classes, res.results).astype(np.float32)


if __name__ == '__main__':
    import time
    t0 = time.time()
    classes = build_all()
    print(f"build: {time.time()-t0:.1f}s")
    for ci, (S, tabs) in enumerate(classes):
        print(f"class{ci}: xlo={S.xlo} xpad={S.xpad} NP={S.NP} pw={S.pw}")
        print(f"  K={S.K.tolist()} cpx={S.cpx} Tn={S.Tn.tolist()} NG={S.NG.tolist()}")
        print(f"  vw_w={S.vw_w} sc1_w={S.sc1_w} sc2_w={S.sc2_w} sc3_w={S.sc3_w} nq={S.nq}")
        tot = sum(v.nbytes for v in tabs[(S.bands[0], 0)].values())
        print(f"  per-core table bytes: {tot/1e6:.1f} MB")
    import importlib.util
    spec = importlib.util.spec_from_file_location("ref", "/root/problem/reference.py")
    ref = importlib.util.module_from_spec(spec)
    spec.loader.exec_module(ref)
    import jax
    with jax.default_device(jax.devices('cpu')[0]):
        inputs = ref.setup_inputs()
        expected = np.asarray(ref.reference(**inputs))
    sgm = np.asarray(inputs['sgm'], np.float32)
    t0 = time.time()
    rec = full_mock(sgm, classes)
    print(f"mock: {time.time()-t0:.1f}s")
    rel = np.abs(rec - expected).max() / np.abs(expected).max()
    print(f"mock rel err: {rel:.4e}")
